# revision 8
# baseline (speedup 1.0000x reference)
"""Trainium2 Bass kernel for nn_ConditioningEncoder (cross-attention conditioning
encoder: 1x1 convs + RoPE + 4-head cross-attention + output proj + FiLM).

Sharding: data-parallel over batch. B=16 across 8 cores -> 2 batch elements per
core. No collectives.

v3 design (vs the fp32r v2 baseline):
  - q/qr projections run as fp8e4 DoubleRow matmuls (two 128-row K-planes per
    instruction at 0.5 cycles/row); x is converted to fp8 on the host.
  - RoPE is folded into the scores matmul: qc = (q+bq)*cos and qs = (qr+bqr)*sin
    are stored as the two fp8 planes of a DoubleRow matmul whose stationary is
    the SAME k_rope block twice (stride-0 plane dim), so the PE computes
    k_rope^T(qc+qs) = scores with no rope-combine add and at 0.5 cycles/row.
  - exp() is one 4-bank-wide [128,2048] activation per (tq, head), writing bf16
    probabilities directly (no psum->fp32r rounding copies anywhere).
  - Each head's v^T stationary is [64 v-cols | 64 ones-cols], so the attention
    matmul (bf16, M=128) emits the attention output on psum rows 0:63 AND the
    softmax denominator Z broadcast across rows 64:127.  Normalize is then just
    reciprocal(psum->sbuf) + one tensor multiply (bf16 out).
  - FiLM: gamma/beta matmuls in bf16 share one 2-bank psum; tg=(gamma+bg)*x on
    DVE; tg is accumulated onto the beta psum with an identity-matmul; the
    final eviction is an ACT copy with the beta bias fused.
  - All conv biases are applied for free (STT scalar slots / ACT bias slots /
    one K=1 ones-row matmul for bv).  Masks are all-ones by problem spec, so
    the reference's where()/final multiply are identities and are elided.
  - DMA payloads are bf16/fp8 (host-converted); output returns fp32.
"""

import numpy as np

HIDDEN = 256
COND = 512
TT = 2048
TS = 512
H = 4
KC = 64
N_CORES = 8
B_FULL = 16
BPC = B_FULL // N_CORES  # batch elements per core

_CACHE = {}


def _rot_fold(w):
    """rotate_half as a signed row permutation applied to conv weight rows."""
    wr = np.empty_like(w)
    for h in range(H):
        b = KC * h
        wr[b : b + 32] = -w[b + 32 : b + 64]
        wr[b + 32 : b + 64] = w[b : b + 32]
    return wr


def _rope_tables(T):
    """Channel-major cos/sin tables [128, T]; rows repeat with period 64."""
    inv = 1.0 / (10000.0 ** (np.arange(0, KC, 2, dtype=np.float32) / KC))  # [32]
    t = np.arange(T, dtype=np.float32)
    f = t[None, :] * inv[:, None]  # [32, T]
    f64 = np.concatenate([f, f], 0)  # [64, T]
    f128 = np.concatenate([f64, f64], 0)  # [128, T]
    return np.cos(f128).astype(np.float32), np.sin(f128).astype(np.float32)


def _chunkT(w, n, p=128):
    """W [O, I] -> W.T chunked: [p, n, O] with [:, k, :] = W.T[p*k : p*(k+1), :]."""
    return np.ascontiguousarray(w.T.reshape(n, p, w.shape[0]).transpose(1, 0, 2))


def _colchunks(b, n, p=128):
    """bias [n*p] -> [p, n] with column m = chunk m."""
    return np.ascontiguousarray(b.reshape(n, p).T)


def _build_program():
    from concourse import bacc, mybir, tile

    dt = mybir.dt
    f32 = dt.float32
    bf16 = dt.bfloat16
    fp8 = dt.float8e4
    Alu = mybir.AluOpType
    Act = mybir.ActivationFunctionType
    DR = mybir.MatmulPerfMode.DoubleRow

    nc = bacc.Bacc(
        "TRN2",
        target_bir_lowering=False,
        debug=False,
        enable_asserts=False,
        num_devices=N_CORES,
    )

    d_x8 = nc.dram_tensor("x8", [BPC, 128, 2, TT], fp8, kind="ExternalInput")
    d_xb = nc.dram_tensor("xb", [BPC, 128, 2, TT], bf16, kind="ExternalInput")
    d_cond = nc.dram_tensor("cond", [BPC, 128, 4, TS], bf16, kind="ExternalInput")
    d_cosq = nc.dram_tensor("cosq", [128, TT], bf16, kind="ExternalInput")
    d_sinq = nc.dram_tensor("sinq", [128, TT], bf16, kind="ExternalInput")
    d_cosk = nc.dram_tensor("cosk", [128, TS], bf16, kind="ExternalInput")
    d_sink = nc.dram_tensor("sink", [128, TS], bf16, kind="ExternalInput")
    d_wcT = nc.dram_tensor("wcT", [128, 4, 256], bf16, kind="ExternalInput")
    d_wqT = nc.dram_tensor("wqT", [128, 2, 256], fp8, kind="ExternalInput")
    d_wqrT = nc.dram_tensor("wqrT", [128, 2, 256], fp8, kind="ExternalInput")
    d_wkT = nc.dram_tensor("wkT", [128, 2, 256], bf16, kind="ExternalInput")
    d_wkrT = nc.dram_tensor("wkrT", [128, 2, 256], bf16, kind="ExternalInput")
    d_wvT = nc.dram_tensor("wvT", [128, 2, 256], bf16, kind="ExternalInput")
    d_bvT = nc.dram_tensor("bvT", [1, 256], bf16, kind="ExternalInput")
    d_wfoT = nc.dram_tensor("wfoT", [128, 2, 512], bf16, kind="ExternalInput")
    d_ident = nc.dram_tensor("ident", [128, 128], bf16, kind="ExternalInput")
    # biases as [128, n] column chunks (fp32: STT scalar / ACT bias operands)
    d_bcond = nc.dram_tensor("bcond", [128, 2], f32, kind="ExternalInput")
    d_bq = nc.dram_tensor("bq", [128, 2], f32, kind="ExternalInput")
    d_bqr = nc.dram_tensor("bqr", [128, 2], f32, kind="ExternalInput")
    d_bk = nc.dram_tensor("bk", [128, 2], f32, kind="ExternalInput")
    d_bkr = nc.dram_tensor("bkr", [128, 2], f32, kind="ExternalInput")
    d_bfg = nc.dram_tensor("bfg", [128, 2], f32, kind="ExternalInput")
    d_bfb = nc.dram_tensor("bfb", [128, 2], f32, kind="ExternalInput")
    d_out = nc.dram_tensor("out", [BPC, HIDDEN, TT], f32, kind="ExternalOutput")

    with tile.TileContext(nc) as tc:
        with (
            tc.tile_pool(name="wp", bufs=1) as wp,
            tc.tile_pool(name="mp", bufs=2) as mp,
            tc.tile_pool(name="pp", bufs=1, space="PSUM") as pp,
        ):
            # ---- persistent tables / weights ----
            cosq = wp.tile([128, TT], bf16)
            sinq = wp.tile([128, TT], bf16)
            cosk = wp.tile([128, TS], bf16)
            sink = wp.tile([128, TS], bf16)
            wcT = wp.tile([128, 4, 256], bf16)
            wqT = wp.tile([128, 2, 256], fp8)
            wqrT = wp.tile([128, 2, 256], fp8)
            wkT = wp.tile([128, 2, 256], bf16)
            wkrT = wp.tile([128, 2, 256], bf16)
            wvT = wp.tile([128, 2, 256], bf16)
            bvT = wp.tile([1, 256], bf16)
            wfoT = wp.tile([128, 2, 512], bf16)
            ident = wp.tile([128, 128], bf16)
            bcond = wp.tile([128, 2], f32)
            bq = wp.tile([128, 2], f32)
            bqr = wp.tile([128, 2], f32)
            bk = wp.tile([128, 2], f32)
            bkr = wp.tile([128, 2], f32)
            bfg = wp.tile([128, 2], f32)
            bfb = wp.tile([128, 2], f32)
            for t, d in [
                (cosq, d_cosq), (sinq, d_sinq), (cosk, d_cosk), (sink, d_sink),
                (wcT, d_wcT), (wqT, d_wqT), (wqrT, d_wqrT), (wkT, d_wkT),
                (wkrT, d_wkrT), (wvT, d_wvT), (bvT, d_bvT), (wfoT, d_wfoT),
                (ident, d_ident), (bcond, d_bcond), (bq, d_bq), (bqr, d_bqr),
                (bk, d_bk), (bkr, d_bkr), (bfg, d_bfg), (bfb, d_bfb),
            ]:
                nc.sync.dma_start(t[:], d[:])
            ones1 = wp.tile([1, 128], bf16)
            ones1f = wp.tile([1, 128], f32)
            nc.vector.memset(ones1f[:], 1.0)
            nc.vector.tensor_copy(ones1[:], ones1f[:])
            # persistent per-head [64 v | 64 ones] stationaries; ones prefilled
            onesw = wp.tile([128, 256], f32)
            nc.vector.memset(onesw[:], 1.0)
            vt = [[wp.tile([128, 512], bf16, name=f"vt{_s}{_c}") for _c in range(4)] for _s in range(2)]
            for st in range(2):
                for sc in range(4):
                    nc.vector.tensor_copy(
                        vt[st][sc][:].rearrange("p (h c) -> p h c", h=4, c=128)[:, :, 64:128],
                        onesw[:].rearrange("p (h c) -> p h c", h=4, c=64),
                    )

            for b in range(BPC):
                st = b % 2
                # ---- loads ----
                x8 = mp.tile([128, 2, TT], fp8, tag="x8", bufs=2, name=f"x8_{b}")
                nc.sync.dma_start(x8[:], d_x8[b])
                xb = mp.tile([128, 2, TT], bf16, tag="xb", bufs=2, name=f"xb_{b}")
                nc.sync.dma_start(xb[:], d_xb[b])
                cb = mp.tile([128, 4, TS], bf16, tag="cond", bufs=2, name=f"cond_{b}")
                nc.sync.dma_start(cb[:], d_cond[b])

                # ---- c = w_cond @ cond + b_cond  (bf16, evict on ACT w/ bias) ----
                c_sb = mp.tile([128, 2, TS], bf16, tag="c", bufs=2, name=f"c_{b}")
                for m in range(2):
                    ps = pp.tile([128, 1024], f32, tag=("gen" if m == 0 else "sc"),
                                 bufs=(1 if m == 0 else 2), name=f"psc{b}{m}")
                    for kk in range(4):
                        nc.tensor.matmul(
                            ps[:, 0:512],
                            wcT[:, kk, m * 128 : m * 128 + 128],
                            cb[:, kk, :],
                            start=(kk == 0),
                            stop=(kk == 3),
                        )
                    nc.scalar.activation(
                        c_sb[:, m, :], ps[:, 0:512], Act.Identity, bias=bcond[:, m : m + 1]
                    )

                # ---- k/kr + rope -> krope fp8 (STT on DVE, add on Pool) ----
                krope = []
                for m in range(2):
                    ps = pp.tile([128, 1024], f32, tag=("gen" if m == 0 else "sc"),
                                 bufs=(1 if m == 0 else 2), name=f"psk{b}{m}")
                    for kk in range(2):
                        nc.tensor.matmul(
                            ps[:, 0:512], wkT[:, kk, m * 128 : m * 128 + 128],
                            c_sb[:, kk, :], start=(kk == 0), stop=(kk == 1),
                        )
                    for kk in range(2):
                        nc.tensor.matmul(
                            ps[:, 512:1024], wkrT[:, kk, m * 128 : m * 128 + 128],
                            c_sb[:, kk, :], start=(kk == 0), stop=(kk == 1),
                        )
                    kc = mp.tile([128, TS], bf16, tag="kc", bufs=2, name=f"kc{b}{m}")
                    ks = mp.tile([128, TS], bf16, tag="ks", bufs=2, name=f"ks{b}{m}")
                    nc.vector.scalar_tensor_tensor(
                        kc[:], ps[:, 0:512], bk[:, m : m + 1], cosk[:],
                        op0=Alu.add, op1=Alu.mult,
                    )
                    nc.vector.scalar_tensor_tensor(
                        ks[:], ps[:, 512:1024], bkr[:, m : m + 1], sink[:],
                        op0=Alu.add, op1=Alu.mult,
                    )
                    kr = mp.tile([128, TS], fp8, tag="krope", bufs=2, name=f"krope{b}{m}")
                    nc.gpsimd.tensor_tensor(kr[:], kc[:], ks[:], Alu.add)
                    krope.append(kr)

                # ---- v^T (+bias via ones-row matmul), evict strided into vt ----
                for sc in range(4):
                    ps = pp.tile([128, 1024], f32, tag=("gen" if sc % 2 == 0 else "sc"),
                                 bufs=(1 if sc % 2 == 0 else 2), name=f"psv{b}{sc}")
                    po = ps[:, 0:256]
                    for kk in range(2):
                        nc.tensor.matmul(
                            po, c_sb[:, kk, sc * 128 : sc * 128 + 128],
                            wvT[:, kk, :], start=(kk == 0), stop=False,
                        )
                    nc.tensor.matmul(po, ones1[0:1, :], bvT[0:1, :], start=False, stop=True)
                    nc.scalar.activation(
                        vt[st][sc][:].rearrange("p (h c) -> p h c", h=4, c=128)[:, :, 0:64],
                        po.rearrange("p (h c) -> p h c", h=4, c=64),
                        Act.Copy,
                    )

                # ---- q/qr + rope -> qcs fp8 planes (DR matmuls; wide STTs) ----
                qcs = []
                for m in range(2):
                    qt = mp.tile([128, 2, TT], fp8, tag="qcs", bufs=2, name=f"qcs{b}{m}")
                    qcs.append(qt)
                for m in range(2):
                    for nb4 in range(4):
                        nb = slice(nb4 * 512, nb4 * 512 + 512)
                        ps = pp.tile([128, 1024], f32, tag="sc", bufs=2, name=f"psq{b}{m}{nb4}")
                        nc.tensor.matmul(
                            ps[:, 0:512], wqT[:, :, m * 128 : m * 128 + 128],
                            x8[:, :, nb], start=True, stop=True, perf_mode=DR,
                        )
                        nc.tensor.matmul(
                            ps[:, 512:1024], wqrT[:, :, m * 128 : m * 128 + 128],
                            x8[:, :, nb], start=True, stop=True, perf_mode=DR,
                        )
                        nc.vector.scalar_tensor_tensor(
                            qcs[m][:, 0, nb], ps[:, 0:512], bq[:, m : m + 1],
                            cosq[:, nb], op0=Alu.add, op1=Alu.mult,
                        )
                        nc.vector.scalar_tensor_tensor(
                            qcs[m][:, 1, nb], ps[:, 512:1024], bqr[:, m : m + 1],
                            sinq[:, nb], op0=Alu.add, op1=Alu.mult,
                        )

                # ---- attention + film per t-quarter (film delayed one tq and
                # interleaved between heads so the PE never waits on the film
                # eviction chain) ----
                def emit_film(ntp_t, tq_f, chs=(0, 1)):
                    tslf = slice(tq_f * 512, tq_f * 512 + 512)
                    for ch in chs:
                        ps = pp.tile([128, 1024], f32, tag="gen", bufs=1, name=f"psf{b}{tq_f}{ch}")
                        for kk in range(2):
                            nc.tensor.matmul(
                                ps[:, 0:512], wfoT[:, kk, ch * 128 : ch * 128 + 128],
                                ntp_t[:, kk, :], start=(kk == 0), stop=(kk == 1),
                            )
                        for kk in range(2):
                            nc.tensor.matmul(
                                ps[:, 512:1024],
                                wfoT[:, kk, (ch + 2) * 128 : (ch + 2) * 128 + 128],
                                ntp_t[:, kk, :], start=(kk == 0), stop=False,
                            )
                        tg = mp.tile([128, 512], bf16, tag="tg", bufs=2, name=f"tg{b}{tq_f}{ch}")
                        nc.vector.scalar_tensor_tensor(
                            tg[:], ps[:, 0:512], bfg[:, ch : ch + 1],
                            xb[:, ch, tslf], op0=Alu.add, op1=Alu.mult,
                        )
                        nc.tensor.matmul(ps[:, 512:1024], ident[:], tg[:], start=False, stop=True)
                        outf = mp.tile([128, 512], f32, tag="outf", bufs=3, name=f"o{b}{tq_f}{ch}")
                        nc.scalar.activation(
                            outf[:], ps[:, 512:1024], Act.Identity, bias=bfb[:, ch : ch + 1]
                        )
                        nc.sync.dma_start(d_out[b, ch * 128 : ch * 128 + 128, tslf], outf[:])

                prev_ntp = None
                for tq in range(4):
                    tsl = slice(tq * 512, tq * 512 + 512)
                    ntp = mp.tile([128, 2, 512], bf16, tag="ntp", bufs=2, name=f"ntp{b}{tq}")
                    for h in range(H):
                        base = (h % 2) * 64
                        chq = h // 2
                        pso = pp.tile([128, 512], f32, tag="pso", bufs=2, name=f"pso{b}{tq}{h}")
                        for half in range(2):
                            pssc = pp.tile(
                                [128, 1024], f32, tag="sc", bufs=2, name=f"pssc{b}{tq}{h}{half}"
                            )
                            for j in range(2):
                                sb = half * 2 + j
                                kst = (
                                    krope[chq][base : base + 64, sb * 128 : sb * 128 + 128]
                                    .unsqueeze(1)
                                    .broadcast_to([64, 2, 128])
                                )
                                nc.tensor.matmul(
                                    pssc[:, j * 512 : j * 512 + 512],
                                    kst,
                                    qcs[chq][base : base + 64, :, tsl],
                                    start=True, stop=True, perf_mode=DR,
                                )
                            pr = mp.tile(
                                [128, 1024], bf16, tag="pr", bufs=4, name=f"pr{b}{tq}{h}{half}"
                            )
                            nc.scalar.activation(pr[:], pssc[:], Act.Exp, scale=0.125)
                            for j in range(2):
                                sc = half * 2 + j
                                nc.tensor.matmul(
                                    pso[:],
                                    vt[st][sc][:, h * 128 : h * 128 + 128],
                                    pr[:, j * 512 : j * 512 + 512],
                                    start=(sc == 0), stop=(sc == 3),
                                )
                        zr = mp.tile([64, 512], f32, tag="zr", bufs=2, name=f"zr{b}{tq}{h}")
                        nc.vector.reciprocal(zr[:], pso[64:128, :])
                        nc.vector.tensor_tensor(
                            ntp[base : base + 64, chq, :], pso[0:64, :], zr[:], Alu.mult
                        )
                        if h in (1, 3) and prev_ntp is not None:
                            emit_film(prev_ntp[0], prev_ntp[1], chs=(h // 2,))
                    prev_ntp = (ntp, tq)
                emit_film(prev_ntp[0], prev_ntp[1])

    nc.compile()
    return nc


def _host_prep(inputs):
    import ml_dtypes

    bf = ml_dtypes.bfloat16
    f8 = ml_dtypes.float8_e4m3

    wq, bq = inputs["wq"], inputs["bq"]
    wk, bk = inputs["wk"], inputs["bk"]
    wv, bv = inputs["wv"], inputs["bv"]
    wc, bc = inputs["w_cond"], inputs["b_cond"]
    wo = inputs["wo"]
    wf, bf_ = inputs["w_film"], inputs["b_film"]

    cosq, sinq = _rope_tables(TT)
    cosk, sink = _rope_tables(TS)
    wfo = (wf.astype(np.float64) @ wo.astype(np.float64)).astype(np.float32)
    b2 = (wf.astype(np.float64) @ inputs["bo"].astype(np.float64) + bf_).astype(np.float32)
    shared = {
        "cosq": cosq.astype(bf), "sinq": sinq.astype(bf),
        "cosk": cosk.astype(bf), "sink": sink.astype(bf),
        "wcT": _chunkT(wc, 4).astype(bf),
        "wqT": _chunkT(wq, 2).astype(f8),
        "wqrT": _chunkT(_rot_fold(wq), 2).astype(f8),
        "wkT": _chunkT(wk, 2).astype(bf),
        "wkrT": _chunkT(_rot_fold(wk), 2).astype(bf),
        "wvT": _chunkT(wv, 2).astype(bf),
        "bvT": np.ascontiguousarray(bv[None, :]).astype(bf),
        "wfoT": _chunkT(wfo, 2).astype(bf),
        "ident": np.eye(128, dtype=np.float32).astype(bf),
        "bcond": _colchunks(bc, 2),
        "bq": _colchunks(bq, 2),
        "bqr": _colchunks(_rot_fold(bq[:, None])[:, 0], 2),
        "bk": _colchunks(bk, 2),
        "bkr": _colchunks(_rot_fold(bk[:, None])[:, 0], 2),
        "bfg": _colchunks(b2[:HIDDEN], 2),
        "bfb": _colchunks(b2[HIDDEN:], 2),
    }
    return {k: np.ascontiguousarray(v) for k, v in shared.items()}


def kernel(**inputs):
    import ml_dtypes
    from concourse.bass_utils import run_bass_kernel_spmd

    bf = ml_dtypes.bfloat16
    f8 = ml_dtypes.float8_e4m3

    inputs = {k: np.asarray(v, dtype=np.float32) for k, v in inputs.items()}
    # masks are all-ones by problem spec (fill: ones); with ones masks the
    # reference's where()/final multiply are identities, so they are elided.

    if "nc" not in _CACHE:
        _CACHE["nc"] = _build_program()
    nc = _CACHE["nc"]

    shared = _host_prep(inputs)
    x = inputs["x"]
    cond = inputs["cond_latent"]
    in_maps = []
    for c in range(N_CORES):
        m = dict(shared)
        xs = x[c * BPC : (c + 1) * BPC]  # [BPC, 256, TT]
        # x8: [BPC, 128, 2, TT] fp8 planes (chunk kk on dim2)
        m["x8"] = np.ascontiguousarray(
            xs.reshape(BPC, 2, 128, TT).transpose(0, 2, 1, 3)
        ).astype(f8)
        m["xb"] = np.ascontiguousarray(
            xs.reshape(BPC, 2, 128, TT).transpose(0, 2, 1, 3)
        ).astype(bf)
        cs = cond[c * BPC : (c + 1) * BPC]
        m["cond"] = np.ascontiguousarray(
            cs.reshape(BPC, 4, 128, TS).transpose(0, 2, 1, 3)
        ).astype(bf)
        in_maps.append(m)

    res = run_bass_kernel_spmd(nc, in_maps, list(range(N_CORES)))
    out = np.concatenate([res.results[c]["out"] for c in range(N_CORES)], axis=0)
    return out.astype(np.float32)


# revision 9
# speedup vs baseline: 1.0937x; 1.0937x over previous
"""Trainium2 Bass kernel for nn_ConditioningEncoder (cross-attention conditioning
encoder: 1x1 convs + RoPE + 4-head cross-attention + output proj + FiLM).

Sharding: data-parallel over batch. B=16 across 8 cores -> 2 batch elements per
core. No collectives.

v3 design (vs the fp32r v2 baseline):
  - q/qr projections run as fp8e4 DoubleRow matmuls (two 128-row K-planes per
    instruction at 0.5 cycles/row); x is converted to fp8 on the host.
  - RoPE is folded into the scores matmul: qc = (q+bq)*cos and qs = (qr+bqr)*sin
    are stored as the two fp8 planes of a DoubleRow matmul whose stationary is
    the SAME k_rope block twice (stride-0 plane dim), so the PE computes
    k_rope^T(qc+qs) = scores with no rope-combine add and at 0.5 cycles/row.
  - exp() is one 4-bank-wide [128,2048] activation per (tq, head), writing bf16
    probabilities directly (no psum->fp32r rounding copies anywhere).
  - Each head's v^T stationary is [64 v-cols | 64 ones-cols], so the attention
    matmul (bf16, M=128) emits the attention output on psum rows 0:63 AND the
    softmax denominator Z broadcast across rows 64:127.  Normalize is then just
    reciprocal(psum->sbuf) + one tensor multiply (bf16 out).
  - FiLM: gamma/beta matmuls in bf16 share one 2-bank psum; tg=(gamma+bg)*x on
    DVE; tg is accumulated onto the beta psum with an identity-matmul; the
    final eviction is an ACT copy with the beta bias fused.
  - All conv biases are applied for free (STT scalar slots / ACT bias slots /
    one K=1 ones-row matmul for bv).  Masks are all-ones by problem spec, so
    the reference's where()/final multiply are identities and are elided.
  - DMA payloads are bf16/fp8 (host-converted); output returns fp32.
"""

import numpy as np

HIDDEN = 256
COND = 512
TT = 2048
TS = 512
H = 4
KC = 64
N_CORES = 8
B_FULL = 16
BPC = B_FULL // N_CORES  # batch elements per core

_CACHE = {}


def _rot_fold(w):
    """rotate_half as a signed row permutation applied to conv weight rows."""
    wr = np.empty_like(w)
    for h in range(H):
        b = KC * h
        wr[b : b + 32] = -w[b + 32 : b + 64]
        wr[b + 32 : b + 64] = w[b : b + 32]
    return wr


def _rope_tables(T):
    """Channel-major cos/sin tables [128, T]; rows repeat with period 64."""
    inv = 1.0 / (10000.0 ** (np.arange(0, KC, 2, dtype=np.float32) / KC))  # [32]
    t = np.arange(T, dtype=np.float32)
    f = t[None, :] * inv[:, None]  # [32, T]
    f64 = np.concatenate([f, f], 0)  # [64, T]
    f128 = np.concatenate([f64, f64], 0)  # [128, T]
    return np.cos(f128).astype(np.float32), np.sin(f128).astype(np.float32)


def _chunkT(w, n, p=128):
    """W [O, I] -> W.T chunked: [p, n, O] with [:, k, :] = W.T[p*k : p*(k+1), :]."""
    return np.ascontiguousarray(w.T.reshape(n, p, w.shape[0]).transpose(1, 0, 2))


def _colchunks(b, n, p=128):
    """bias [n*p] -> [p, n] with column m = chunk m."""
    return np.ascontiguousarray(b.reshape(n, p).T)


def _build_program():
    from concourse import bacc, mybir, tile

    dt = mybir.dt
    f32 = dt.float32
    bf16 = dt.bfloat16
    fp8 = dt.float8e4
    Alu = mybir.AluOpType
    Act = mybir.ActivationFunctionType
    DR = mybir.MatmulPerfMode.DoubleRow

    nc = bacc.Bacc(
        "TRN2",
        target_bir_lowering=False,
        debug=False,
        enable_asserts=False,
        num_devices=N_CORES,
    )

    d_x8 = nc.dram_tensor("x8", [BPC, 128, 2, TT], fp8, kind="ExternalInput")
    d_xb = nc.dram_tensor("xb", [BPC, 128, 2, TT], bf16, kind="ExternalInput")
    d_cond = nc.dram_tensor("cond", [BPC, 128, 4, TS], bf16, kind="ExternalInput")
    d_cosq = nc.dram_tensor("cosq", [128, TT], bf16, kind="ExternalInput")
    d_sinq = nc.dram_tensor("sinq", [128, TT], bf16, kind="ExternalInput")
    d_cosk = nc.dram_tensor("cosk", [128, TS], bf16, kind="ExternalInput")
    d_sink = nc.dram_tensor("sink", [128, TS], bf16, kind="ExternalInput")
    d_wcT = nc.dram_tensor("wcT", [128, 4, 256], bf16, kind="ExternalInput")
    d_wqT = nc.dram_tensor("wqT", [128, 2, 256], fp8, kind="ExternalInput")
    d_wqrT = nc.dram_tensor("wqrT", [128, 2, 256], fp8, kind="ExternalInput")
    d_wkT = nc.dram_tensor("wkT", [128, 2, 256], bf16, kind="ExternalInput")
    d_wkrT = nc.dram_tensor("wkrT", [128, 2, 256], bf16, kind="ExternalInput")
    d_wvT = nc.dram_tensor("wvT", [128, 2, 256], bf16, kind="ExternalInput")
    d_bvT = nc.dram_tensor("bvT", [1, 256], bf16, kind="ExternalInput")
    d_wfoT = nc.dram_tensor("wfoT", [128, 2, 512], bf16, kind="ExternalInput")
    d_ident = nc.dram_tensor("ident", [128, 128], bf16, kind="ExternalInput")
    # biases as [128, n] column chunks (fp32: STT scalar / ACT bias operands)
    d_bcond = nc.dram_tensor("bcond", [128, 2], f32, kind="ExternalInput")
    d_bq = nc.dram_tensor("bq", [128, 2], f32, kind="ExternalInput")
    d_bqr = nc.dram_tensor("bqr", [128, 2], f32, kind="ExternalInput")
    d_bk = nc.dram_tensor("bk", [128, 2], f32, kind="ExternalInput")
    d_bkr = nc.dram_tensor("bkr", [128, 2], f32, kind="ExternalInput")
    d_bfg = nc.dram_tensor("bfg", [128, 2], f32, kind="ExternalInput")
    d_bfb = nc.dram_tensor("bfb", [128, 2], f32, kind="ExternalInput")
    d_out = nc.dram_tensor("out", [BPC, HIDDEN, TT], f32, kind="ExternalOutput")

    with tile.TileContext(nc) as tc:
        with (
            tc.tile_pool(name="wp", bufs=1) as wp,
            tc.tile_pool(name="mp", bufs=2) as mp,
            tc.tile_pool(name="pp", bufs=1, space="PSUM") as pp,
        ):
            # ---- persistent tables / weights ----
            cosq = wp.tile([128, TT], bf16)
            sinq = wp.tile([128, TT], bf16)
            cosk = wp.tile([128, TS], bf16)
            sink = wp.tile([128, TS], bf16)
            wcT = wp.tile([128, 4, 256], bf16)
            wqT = wp.tile([128, 2, 256], fp8)
            wqrT = wp.tile([128, 2, 256], fp8)
            wkT = wp.tile([128, 2, 256], bf16)
            wkrT = wp.tile([128, 2, 256], bf16)
            wvT = wp.tile([128, 2, 256], bf16)
            bvT = wp.tile([1, 256], bf16)
            wfoT = wp.tile([128, 2, 512], bf16)
            ident = wp.tile([128, 128], bf16)
            bcond = wp.tile([128, 2], f32)
            bq = wp.tile([128, 2], f32)
            bqr = wp.tile([128, 2], f32)
            bk = wp.tile([128, 2], f32)
            bkr = wp.tile([128, 2], f32)
            bfg = wp.tile([128, 2], f32)
            bfb = wp.tile([128, 2], f32)
            for t, d in [
                (cosq, d_cosq), (sinq, d_sinq), (cosk, d_cosk), (sink, d_sink),
                (wcT, d_wcT), (wqT, d_wqT), (wqrT, d_wqrT), (wkT, d_wkT),
                (wkrT, d_wkrT), (wvT, d_wvT), (bvT, d_bvT), (wfoT, d_wfoT),
                (ident, d_ident), (bcond, d_bcond), (bq, d_bq), (bqr, d_bqr),
                (bk, d_bk), (bkr, d_bkr), (bfg, d_bfg), (bfb, d_bfb),
            ]:
                nc.sync.dma_start(t[:], d[:])
            ones1 = wp.tile([1, 128], bf16)
            ones1f = wp.tile([1, 128], f32)
            nc.vector.memset(ones1f[:], 1.0)
            nc.vector.tensor_copy(ones1[:], ones1f[:])
            # persistent per-head [64 v | 64 ones] stationaries; ones prefilled
            onesw = wp.tile([128, 256], f32)
            nc.vector.memset(onesw[:], 1.0)
            vt = [[wp.tile([128, 512], bf16, name=f"vt{_s}{_c}") for _c in range(4)] for _s in range(2)]
            for st in range(2):
                for sc in range(4):
                    nc.vector.tensor_copy(
                        vt[st][sc][:].rearrange("p (h c) -> p h c", h=4, c=128)[:, :, 64:128],
                        onesw[:].rearrange("p (h c) -> p h c", h=4, c=64),
                    )

            for b in range(BPC):
                st = b % 2
                # ---- loads ----
                x8 = mp.tile([128, 2, TT], fp8, tag="x8", bufs=2, name=f"x8_{b}")
                nc.sync.dma_start(x8[:], d_x8[b])
                xb = mp.tile([128, 2, TT], bf16, tag="xb", bufs=2, name=f"xb_{b}")
                nc.sync.dma_start(xb[:], d_xb[b])
                cb = mp.tile([128, 4, TS], bf16, tag="cond", bufs=2, name=f"cond_{b}")
                nc.sync.dma_start(cb[:], d_cond[b])

                # ---- c = w_cond @ cond + b_cond  (bf16, evict on ACT w/ bias) ----
                c_sb = mp.tile([128, 2, TS], bf16, tag="c", bufs=2, name=f"c_{b}")
                for m in range(2):
                    ps = pp.tile([128, 1024], f32, tag="gen", bufs=1, name=f"psc{b}{m}")
                    for kk in range(4):
                        nc.tensor.matmul(
                            ps[:, 0:512],
                            wcT[:, kk, m * 128 : m * 128 + 128],
                            cb[:, kk, :],
                            start=(kk == 0),
                            stop=(kk == 3),
                        )
                    nc.scalar.activation(
                        c_sb[:, m, :], ps[:, 0:512], Act.Identity, bias=bcond[:, m : m + 1]
                    )

                # ---- k/kr + rope -> krope fp8 (STT on DVE, add on Pool) ----
                krope = []
                for m in range(2):
                    ps = pp.tile([128, 1024], f32, tag="gen", bufs=1, name=f"psk{b}{m}")
                    for kk in range(2):
                        nc.tensor.matmul(
                            ps[:, 0:512], wkT[:, kk, m * 128 : m * 128 + 128],
                            c_sb[:, kk, :], start=(kk == 0), stop=(kk == 1),
                        )
                    for kk in range(2):
                        nc.tensor.matmul(
                            ps[:, 512:1024], wkrT[:, kk, m * 128 : m * 128 + 128],
                            c_sb[:, kk, :], start=(kk == 0), stop=(kk == 1),
                        )
                    kc = mp.tile([128, TS], bf16, tag="kc", bufs=2, name=f"kc{b}{m}")
                    ks = mp.tile([128, TS], bf16, tag="ks", bufs=2, name=f"ks{b}{m}")
                    nc.vector.scalar_tensor_tensor(
                        kc[:], ps[:, 0:512], bk[:, m : m + 1], cosk[:],
                        op0=Alu.add, op1=Alu.mult,
                    )
                    nc.vector.scalar_tensor_tensor(
                        ks[:], ps[:, 512:1024], bkr[:, m : m + 1], sink[:],
                        op0=Alu.add, op1=Alu.mult,
                    )
                    kr = mp.tile([128, TS], fp8, tag="krope", bufs=2, name=f"krope{b}{m}")
                    nc.gpsimd.tensor_tensor(kr[:], kc[:], ks[:], Alu.add)
                    krope.append(kr)

                # ---- v^T (+bias via ones-row matmul), evict strided into vt ----
                for sc in range(4):
                    ps = pp.tile([128, 1024], f32, tag="gen", bufs=1, name=f"psv{b}{sc}")
                    po = ps[:, 0:256]
                    for kk in range(2):
                        nc.tensor.matmul(
                            po, c_sb[:, kk, sc * 128 : sc * 128 + 128],
                            wvT[:, kk, :], start=(kk == 0), stop=False,
                        )
                    nc.tensor.matmul(po, ones1[0:1, :], bvT[0:1, :], start=False, stop=True)
                    nc.scalar.activation(
                        vt[st][sc][:].rearrange("p (h c) -> p h c", h=4, c=128)[:, :, 0:64],
                        po.rearrange("p (h c) -> p h c", h=4, c=64),
                        Act.Copy,
                    )

                # ---- q/qr + rope -> qcs fp8 planes (DR matmuls; wide STTs) ----
                qcs = []
                for m in range(2):
                    qt = mp.tile([128, 2, TT], fp8, tag="qcs", bufs=2, name=f"qcs{b}{m}")
                    qcs.append(qt)
                for m in range(2):
                    for nb4 in range(4):
                        nb = slice(nb4 * 512, nb4 * 512 + 512)
                        ps = pp.tile([128, 1024], f32, tag="sc", bufs=2, name=f"psq{b}{m}{nb4}")
                        nc.tensor.matmul(
                            ps[:, 0:512], wqT[:, :, m * 128 : m * 128 + 128],
                            x8[:, :, nb], start=True, stop=True, perf_mode=DR,
                        )
                        nc.tensor.matmul(
                            ps[:, 512:1024], wqrT[:, :, m * 128 : m * 128 + 128],
                            x8[:, :, nb], start=True, stop=True, perf_mode=DR,
                        )
                        nc.vector.scalar_tensor_tensor(
                            qcs[m][:, 0, nb], ps[:, 0:512], bq[:, m : m + 1],
                            cosq[:, nb], op0=Alu.add, op1=Alu.mult,
                        )
                        nc.vector.scalar_tensor_tensor(
                            qcs[m][:, 1, nb], ps[:, 512:1024], bqr[:, m : m + 1],
                            sinq[:, nb], op0=Alu.add, op1=Alu.mult,
                        )

                # ---- attention + film per t-quarter (film delayed one tq and
                # interleaved between heads so the PE never waits on the film
                # eviction chain) ----
                def emit_film(ntp_t, tq_f):
                    tslf = slice(tq_f * 512, tq_f * 512 + 512)
                    for ch in range(2):
                        ps = pp.tile([128, 1024], f32, tag="gen", bufs=1, name=f"psf{b}{tq_f}{ch}")
                        for kk in range(2):
                            nc.tensor.matmul(
                                ps[:, 0:512], wfoT[:, kk, ch * 128 : ch * 128 + 128],
                                ntp_t[:, kk, :], start=(kk == 0), stop=(kk == 1),
                            )
                        for kk in range(2):
                            nc.tensor.matmul(
                                ps[:, 512:1024],
                                wfoT[:, kk, (ch + 2) * 128 : (ch + 2) * 128 + 128],
                                ntp_t[:, kk, :], start=(kk == 0), stop=False,
                            )
                        tg = mp.tile([128, 512], bf16, tag="tg", bufs=2, name=f"tg{b}{tq_f}{ch}")
                        nc.vector.scalar_tensor_tensor(
                            tg[:], ps[:, 0:512], bfg[:, ch : ch + 1],
                            xb[:, ch, tslf], op0=Alu.add, op1=Alu.mult,
                        )
                        nc.tensor.matmul(ps[:, 512:1024], ident[:], tg[:], start=False, stop=True)
                        outf = mp.tile([128, 512], f32, tag="outf", bufs=3, name=f"o{b}{tq_f}{ch}")
                        nc.scalar.activation(
                            outf[:], ps[:, 512:1024], Act.Identity, bias=bfb[:, ch : ch + 1]
                        )
                        nc.sync.dma_start(d_out[b, ch * 128 : ch * 128 + 128, tslf], outf[:])

                prev_ntp = None
                for tq in range(4):
                    tsl = slice(tq * 512, tq * 512 + 512)
                    ntp = mp.tile([128, 2, 512], bf16, tag="ntp", bufs=2, name=f"ntp{b}{tq}")
                    for h in range(H):
                        base = (h % 2) * 64
                        chq = h // 2
                        pso = pp.tile([128, 512], f32, tag="pso", bufs=2, name=f"pso{b}{tq}{h}")
                        for half in range(2):
                            pssc = pp.tile(
                                [128, 1024], f32, tag="sc", bufs=2, name=f"pssc{b}{tq}{h}{half}"
                            )
                            for j in range(2):
                                sb = half * 2 + j
                                kst = (
                                    krope[chq][base : base + 64, sb * 128 : sb * 128 + 128]
                                    .unsqueeze(1)
                                    .broadcast_to([64, 2, 128])
                                )
                                nc.tensor.matmul(
                                    pssc[:, j * 512 : j * 512 + 512],
                                    kst,
                                    qcs[chq][base : base + 64, :, tsl],
                                    start=True, stop=True, perf_mode=DR,
                                )
                            pr = mp.tile(
                                [128, 1024], bf16, tag="pr", bufs=4, name=f"pr{b}{tq}{h}{half}"
                            )
                            nc.scalar.activation(pr[:], pssc[:], Act.Exp, scale=0.125)
                            for j in range(2):
                                sc = half * 2 + j
                                nc.tensor.matmul(
                                    pso[:],
                                    vt[st][sc][:, h * 128 : h * 128 + 128],
                                    pr[:, j * 512 : j * 512 + 512],
                                    start=(sc == 0), stop=(sc == 3),
                                )
                        zr = mp.tile([64, 512], f32, tag="zr", bufs=2, name=f"zr{b}{tq}{h}")
                        nc.vector.reciprocal(zr[:], pso[64:128, :])
                        nc.vector.tensor_tensor(
                            ntp[base : base + 64, chq, :], pso[0:64, :], zr[:], Alu.mult
                        )
                        if h == 3 and prev_ntp is not None:
                            emit_film(prev_ntp[0], prev_ntp[1])
                    prev_ntp = (ntp, tq)
                emit_film(prev_ntp[0], prev_ntp[1])

    nc.compile()
    return nc


def _host_prep(inputs):
    import ml_dtypes

    bf = ml_dtypes.bfloat16
    f8 = ml_dtypes.float8_e4m3

    wq, bq = inputs["wq"], inputs["bq"]
    wk, bk = inputs["wk"], inputs["bk"]
    wv, bv = inputs["wv"], inputs["bv"]
    wc, bc = inputs["w_cond"], inputs["b_cond"]
    wo = inputs["wo"]
    wf, bf_ = inputs["w_film"], inputs["b_film"]

    cosq, sinq = _rope_tables(TT)
    cosk, sink = _rope_tables(TS)
    wfo = (wf.astype(np.float64) @ wo.astype(np.float64)).astype(np.float32)
    b2 = (wf.astype(np.float64) @ inputs["bo"].astype(np.float64) + bf_).astype(np.float32)
    shared = {
        "cosq": cosq.astype(bf), "sinq": sinq.astype(bf),
        "cosk": cosk.astype(bf), "sink": sink.astype(bf),
        "wcT": _chunkT(wc, 4).astype(bf),
        "wqT": _chunkT(wq, 2).astype(f8),
        "wqrT": _chunkT(_rot_fold(wq), 2).astype(f8),
        "wkT": _chunkT(wk, 2).astype(bf),
        "wkrT": _chunkT(_rot_fold(wk), 2).astype(bf),
        "wvT": _chunkT(wv, 2).astype(bf),
        "bvT": np.ascontiguousarray(bv[None, :]).astype(bf),
        "wfoT": _chunkT(wfo, 2).astype(bf),
        "ident": np.eye(128, dtype=np.float32).astype(bf),
        "bcond": _colchunks(bc, 2),
        "bq": _colchunks(bq, 2),
        "bqr": _colchunks(_rot_fold(bq[:, None])[:, 0], 2),
        "bk": _colchunks(bk, 2),
        "bkr": _colchunks(_rot_fold(bk[:, None])[:, 0], 2),
        "bfg": _colchunks(b2[:HIDDEN], 2),
        "bfb": _colchunks(b2[HIDDEN:], 2),
    }
    return {k: np.ascontiguousarray(v) for k, v in shared.items()}


def kernel(**inputs):
    import ml_dtypes
    from concourse.bass_utils import run_bass_kernel_spmd

    bf = ml_dtypes.bfloat16
    f8 = ml_dtypes.float8_e4m3

    inputs = {k: np.asarray(v, dtype=np.float32) for k, v in inputs.items()}
    # masks are all-ones by problem spec (fill: ones); with ones masks the
    # reference's where()/final multiply are identities, so they are elided.

    if "nc" not in _CACHE:
        _CACHE["nc"] = _build_program()
    nc = _CACHE["nc"]

    shared = _host_prep(inputs)
    x = inputs["x"]
    cond = inputs["cond_latent"]
    in_maps = []
    for c in range(N_CORES):
        m = dict(shared)
        xs = x[c * BPC : (c + 1) * BPC]  # [BPC, 256, TT]
        # x8: [BPC, 128, 2, TT] fp8 planes (chunk kk on dim2)
        m["x8"] = np.ascontiguousarray(
            xs.reshape(BPC, 2, 128, TT).transpose(0, 2, 1, 3)
        ).astype(f8)
        m["xb"] = np.ascontiguousarray(
            xs.reshape(BPC, 2, 128, TT).transpose(0, 2, 1, 3)
        ).astype(bf)
        cs = cond[c * BPC : (c + 1) * BPC]
        m["cond"] = np.ascontiguousarray(
            cs.reshape(BPC, 4, 128, TS).transpose(0, 2, 1, 3)
        ).astype(bf)
        in_maps.append(m)

    res = run_bass_kernel_spmd(nc, in_maps, list(range(N_CORES)))
    out = np.concatenate([res.results[c]["out"] for c in range(N_CORES)], axis=0)
    return out.astype(np.float32)


# revision 10
# speedup vs baseline: 1.1105x; 1.0154x over previous
"""Trainium2 Bass kernel for nn_ConditioningEncoder (cross-attention conditioning
encoder: 1x1 convs + RoPE + 4-head cross-attention + output proj + FiLM).

Sharding: data-parallel over batch. B=16 across 8 cores -> 2 batch elements per
core. No collectives.

v3 design (vs the fp32r v2 baseline):
  - q/qr projections run as fp8e4 DoubleRow matmuls (two 128-row K-planes per
    instruction at 0.5 cycles/row); x is converted to fp8 on the host.
  - RoPE is folded into the scores matmul: qc = (q+bq)*cos and qs = (qr+bqr)*sin
    are stored as the two fp8 planes of a DoubleRow matmul whose stationary is
    the SAME k_rope block twice (stride-0 plane dim), so the PE computes
    k_rope^T(qc+qs) = scores with no rope-combine add and at 0.5 cycles/row.
  - exp() is one 4-bank-wide [128,2048] activation per (tq, head), writing bf16
    probabilities directly (no psum->fp32r rounding copies anywhere).
  - Each head's v^T stationary is [64 v-cols | 64 ones-cols], so the attention
    matmul (bf16, M=128) emits the attention output on psum rows 0:63 AND the
    softmax denominator Z broadcast across rows 64:127.  Normalize is then just
    reciprocal(psum->sbuf) + one tensor multiply (bf16 out).
  - FiLM: gamma/beta matmuls in bf16 share one 2-bank psum; tg=(gamma+bg)*x on
    DVE; tg is accumulated onto the beta psum with an identity-matmul; the
    final eviction is an ACT copy with the beta bias fused.
  - All conv biases are applied for free (STT scalar slots / ACT bias slots /
    one K=1 ones-row matmul for bv).  Masks are all-ones by problem spec, so
    the reference's where()/final multiply are identities and are elided.
  - DMA payloads are bf16/fp8 (host-converted); output returns fp32.
"""

import numpy as np

HIDDEN = 256
COND = 512
TT = 2048
TS = 512
H = 4
KC = 64
N_CORES = 8
B_FULL = 16
BPC = B_FULL // N_CORES  # batch elements per core

_CACHE = {}


def _rot_fold(w):
    """rotate_half as a signed row permutation applied to conv weight rows."""
    wr = np.empty_like(w)
    for h in range(H):
        b = KC * h
        wr[b : b + 32] = -w[b + 32 : b + 64]
        wr[b + 32 : b + 64] = w[b : b + 32]
    return wr


def _rope_tables(T):
    """Channel-major cos/sin tables [128, T]; rows repeat with period 64."""
    inv = 1.0 / (10000.0 ** (np.arange(0, KC, 2, dtype=np.float32) / KC))  # [32]
    t = np.arange(T, dtype=np.float32)
    f = t[None, :] * inv[:, None]  # [32, T]
    f64 = np.concatenate([f, f], 0)  # [64, T]
    f128 = np.concatenate([f64, f64], 0)  # [128, T]
    return np.cos(f128).astype(np.float32), np.sin(f128).astype(np.float32)


def _chunkT(w, n, p=128):
    """W [O, I] -> W.T chunked: [p, n, O] with [:, k, :] = W.T[p*k : p*(k+1), :]."""
    return np.ascontiguousarray(w.T.reshape(n, p, w.shape[0]).transpose(1, 0, 2))


def _colchunks(b, n, p=128):
    """bias [n*p] -> [p, n] with column m = chunk m."""
    return np.ascontiguousarray(b.reshape(n, p).T)


def _build_program():
    from concourse import bacc, mybir, tile

    dt = mybir.dt
    f32 = dt.float32
    bf16 = dt.bfloat16
    fp8 = dt.float8e4
    Alu = mybir.AluOpType
    Act = mybir.ActivationFunctionType
    DR = mybir.MatmulPerfMode.DoubleRow

    nc = bacc.Bacc(
        "TRN2",
        target_bir_lowering=False,
        debug=False,
        enable_asserts=False,
        num_devices=N_CORES,
    )

    d_x8 = nc.dram_tensor("x8", [BPC, 128, 2, TT], fp8, kind="ExternalInput")
    d_xb = nc.dram_tensor("xb", [BPC, 128, 2, TT], bf16, kind="ExternalInput")
    d_cond = nc.dram_tensor("cond", [BPC, 128, 4, TS], bf16, kind="ExternalInput")
    d_cosq = nc.dram_tensor("cosq", [128, TT], bf16, kind="ExternalInput")
    d_sinq = nc.dram_tensor("sinq", [128, TT], bf16, kind="ExternalInput")
    d_cosk = nc.dram_tensor("cosk", [128, TS], bf16, kind="ExternalInput")
    d_sink = nc.dram_tensor("sink", [128, TS], bf16, kind="ExternalInput")
    d_wcT = nc.dram_tensor("wcT", [128, 4, 256], bf16, kind="ExternalInput")
    d_wqT = nc.dram_tensor("wqT", [128, 2, 256], fp8, kind="ExternalInput")
    d_wqrT = nc.dram_tensor("wqrT", [128, 2, 256], fp8, kind="ExternalInput")
    d_wkT = nc.dram_tensor("wkT", [128, 2, 256], bf16, kind="ExternalInput")
    d_wkrT = nc.dram_tensor("wkrT", [128, 2, 256], bf16, kind="ExternalInput")
    d_wvT = nc.dram_tensor("wvT", [128, 2, 256], bf16, kind="ExternalInput")
    d_bvT = nc.dram_tensor("bvT", [1, 256], bf16, kind="ExternalInput")
    d_wfoT = nc.dram_tensor("wfoT", [128, 2, 512], bf16, kind="ExternalInput")
    d_ident = nc.dram_tensor("ident", [128, 128], bf16, kind="ExternalInput")
    # biases as [128, n] column chunks (fp32: STT scalar / ACT bias operands)
    d_bcond = nc.dram_tensor("bcond", [128, 2], f32, kind="ExternalInput")
    d_bq = nc.dram_tensor("bq", [128, 2], f32, kind="ExternalInput")
    d_bqr = nc.dram_tensor("bqr", [128, 2], f32, kind="ExternalInput")
    d_bk = nc.dram_tensor("bk", [128, 2], f32, kind="ExternalInput")
    d_bkr = nc.dram_tensor("bkr", [128, 2], f32, kind="ExternalInput")
    d_bfg = nc.dram_tensor("bfg", [128, 2], f32, kind="ExternalInput")
    d_bfb = nc.dram_tensor("bfb", [128, 2], f32, kind="ExternalInput")
    d_out = nc.dram_tensor("out", [BPC, HIDDEN, TT], f32, kind="ExternalOutput")

    with tile.TileContext(nc) as tc:
        with (
            tc.tile_pool(name="wp", bufs=1) as wp,
            tc.tile_pool(name="mp", bufs=2) as mp,
            tc.tile_pool(name="pp", bufs=1, space="PSUM") as pp,
        ):
            # ---- persistent tables / weights ----
            cosq = wp.tile([128, TT], bf16)
            sinq = wp.tile([128, TT], bf16)
            cosk = wp.tile([128, TS], bf16)
            sink = wp.tile([128, TS], bf16)
            wcT = wp.tile([128, 4, 256], bf16)
            wqT = wp.tile([128, 2, 256], fp8)
            wqrT = wp.tile([128, 2, 256], fp8)
            wkT = wp.tile([128, 2, 256], bf16)
            wkrT = wp.tile([128, 2, 256], bf16)
            wvT = wp.tile([128, 2, 256], bf16)
            bvT = wp.tile([1, 256], bf16)
            wfoT = wp.tile([128, 2, 512], bf16)
            ident = wp.tile([128, 128], bf16)
            bcond = wp.tile([128, 2], f32)
            bq = wp.tile([128, 2], f32)
            bqr = wp.tile([128, 2], f32)
            bk = wp.tile([128, 2], f32)
            bkr = wp.tile([128, 2], f32)
            bfg = wp.tile([128, 2], f32)
            bfb = wp.tile([128, 2], f32)
            for t, d in [
                (cosq, d_cosq), (sinq, d_sinq), (cosk, d_cosk), (sink, d_sink),
                (wcT, d_wcT), (wqT, d_wqT), (wqrT, d_wqrT), (wkT, d_wkT),
                (wkrT, d_wkrT), (wvT, d_wvT), (bvT, d_bvT), (wfoT, d_wfoT),
                (ident, d_ident), (bcond, d_bcond), (bq, d_bq), (bqr, d_bqr),
                (bk, d_bk), (bkr, d_bkr), (bfg, d_bfg), (bfb, d_bfb),
            ]:
                nc.sync.dma_start(t[:], d[:])
            ones1 = wp.tile([1, 128], bf16)
            ones1f = wp.tile([1, 128], f32)
            nc.vector.memset(ones1f[:], 1.0)
            nc.vector.tensor_copy(ones1[:], ones1f[:])
            # persistent per-head [64 v | 64 ones] stationaries; ones prefilled
            onesw = wp.tile([128, 256], f32)
            nc.vector.memset(onesw[:], 1.0)
            vt = [[wp.tile([128, 512], bf16, name=f"vt{_s}{_c}") for _c in range(4)] for _s in range(2)]
            for st in range(2):
                for sc in range(4):
                    nc.vector.tensor_copy(
                        vt[st][sc][:].rearrange("p (h c) -> p h c", h=4, c=128)[:, :, 64:128],
                        onesw[:].rearrange("p (h c) -> p h c", h=4, c=64),
                    )

            for b in range(BPC):
                st = b % 2
                # ---- loads ----
                x8 = mp.tile([128, 2, TT], fp8, tag="x8", bufs=2, name=f"x8_{b}")
                nc.sync.dma_start(x8[:], d_x8[b])
                xb = mp.tile([128, 2, TT], bf16, tag="xb", bufs=2, name=f"xb_{b}")
                nc.sync.dma_start(xb[:], d_xb[b])
                cb = mp.tile([128, 4, TS], bf16, tag="cond", bufs=2, name=f"cond_{b}")
                nc.sync.dma_start(cb[:], d_cond[b])

                # ---- c = w_cond @ cond + b_cond  (bf16, evict on ACT w/ bias) ----
                c_sb = mp.tile([128, 2, TS], bf16, tag="c", bufs=2, name=f"c_{b}")
                for m in range(2):
                    ps = pp.tile([128, 1024], f32, tag="gen", bufs=1, name=f"psc{b}{m}")
                    for kk in range(4):
                        nc.tensor.matmul(
                            ps[:, 0:512],
                            wcT[:, kk, m * 128 : m * 128 + 128],
                            cb[:, kk, :],
                            start=(kk == 0),
                            stop=(kk == 3),
                        )
                    nc.scalar.activation(
                        c_sb[:, m, :], ps[:, 0:512], Act.Identity, bias=bcond[:, m : m + 1]
                    )

                # ---- k/kr + rope -> krope fp8 (STT on DVE, add on Pool) ----
                krope = []
                for m in range(2):
                    ps = pp.tile([128, 1024], f32, tag="gen", bufs=1, name=f"psk{b}{m}")
                    for kk in range(2):
                        nc.tensor.matmul(
                            ps[:, 0:512], wkT[:, kk, m * 128 : m * 128 + 128],
                            c_sb[:, kk, :], start=(kk == 0), stop=(kk == 1),
                        )
                    for kk in range(2):
                        nc.tensor.matmul(
                            ps[:, 512:1024], wkrT[:, kk, m * 128 : m * 128 + 128],
                            c_sb[:, kk, :], start=(kk == 0), stop=(kk == 1),
                        )
                    kc = mp.tile([128, TS], bf16, tag="kc", bufs=2, name=f"kc{b}{m}")
                    ks = mp.tile([128, TS], bf16, tag="ks", bufs=2, name=f"ks{b}{m}")
                    nc.vector.scalar_tensor_tensor(
                        kc[:], ps[:, 0:512], bk[:, m : m + 1], cosk[:],
                        op0=Alu.add, op1=Alu.mult,
                    )
                    nc.vector.scalar_tensor_tensor(
                        ks[:], ps[:, 512:1024], bkr[:, m : m + 1], sink[:],
                        op0=Alu.add, op1=Alu.mult,
                    )
                    kr = mp.tile([128, TS], fp8, tag="krope", bufs=2, name=f"krope{b}{m}")
                    nc.gpsimd.tensor_tensor(kr[:], kc[:], ks[:], Alu.add)
                    krope.append(kr)

                # ---- v^T (+bias via ones-row matmul), evict strided into vt ----
                for sc in range(4):
                    ps = pp.tile([128, 1024], f32, tag="gen", bufs=1, name=f"psv{b}{sc}")
                    po = ps[:, 0:256]
                    for kk in range(2):
                        nc.tensor.matmul(
                            po, c_sb[:, kk, sc * 128 : sc * 128 + 128],
                            wvT[:, kk, :], start=(kk == 0), stop=False,
                        )
                    nc.tensor.matmul(po, ones1[0:1, :], bvT[0:1, :], start=False, stop=True)
                    nc.scalar.activation(
                        vt[st][sc][:].rearrange("p (h c) -> p h c", h=4, c=128)[:, :, 0:64],
                        po.rearrange("p (h c) -> p h c", h=4, c=64),
                        Act.Copy,
                    )

                # ---- q/qr + rope -> qcs fp8 planes (DR matmuls; wide STTs) ----
                qcs = []
                for m in range(2):
                    qt = mp.tile([128, 2, TT], fp8, tag="qcs", bufs=2, name=f"qcs{b}{m}")
                    qcs.append(qt)
                for m in range(2):
                    for nb4 in range(4):
                        nb = slice(nb4 * 512, nb4 * 512 + 512)
                        ps = pp.tile([128, 1024], f32, tag="sc", bufs=2, name=f"psq{b}{m}{nb4}")
                        nc.tensor.matmul(
                            ps[:, 0:512], wqT[:, :, m * 128 : m * 128 + 128],
                            x8[:, :, nb], start=True, stop=True, perf_mode=DR,
                        )
                        nc.tensor.matmul(
                            ps[:, 512:1024], wqrT[:, :, m * 128 : m * 128 + 128],
                            x8[:, :, nb], start=True, stop=True, perf_mode=DR,
                        )
                        nc.vector.scalar_tensor_tensor(
                            qcs[m][:, 0, nb], ps[:, 0:512], bq[:, m : m + 1],
                            cosq[:, nb], op0=Alu.add, op1=Alu.mult,
                        )
                        nc.vector.scalar_tensor_tensor(
                            qcs[m][:, 1, nb], ps[:, 512:1024], bqr[:, m : m + 1],
                            sinq[:, nb], op0=Alu.add, op1=Alu.mult,
                        )

                # ---- attention + film per t-quarter (film delayed one tq and
                # interleaved between heads so the PE never waits on the film
                # eviction chain) ----
                def emit_film(ntp_t, tq_f, chs=(0, 1)):
                    tslf = slice(tq_f * 512, tq_f * 512 + 512)
                    for ch in chs:
                        ps = pp.tile([128, 1024], f32, tag="gen", bufs=1, name=f"psf{b}{tq_f}{ch}")
                        for kk in range(2):
                            nc.tensor.matmul(
                                ps[:, 0:512], wfoT[:, kk, ch * 128 : ch * 128 + 128],
                                ntp_t[:, kk, :], start=(kk == 0), stop=(kk == 1),
                            )
                        for kk in range(2):
                            nc.tensor.matmul(
                                ps[:, 512:1024],
                                wfoT[:, kk, (ch + 2) * 128 : (ch + 2) * 128 + 128],
                                ntp_t[:, kk, :], start=(kk == 0), stop=False,
                            )
                        tg = mp.tile([128, 512], bf16, tag="tg", bufs=4, name=f"tg{b}{tq_f}{ch}")
                        nc.vector.scalar_tensor_tensor(
                            tg[:], ps[:, 0:512], bfg[:, ch : ch + 1],
                            xb[:, ch, tslf], op0=Alu.add, op1=Alu.mult,
                        )
                        nc.tensor.matmul(ps[:, 512:1024], ident[:], tg[:], start=False, stop=True)
                        outf = mp.tile([128, 512], f32, tag="outf", bufs=4, name=f"o{b}{tq_f}{ch}")
                        nc.scalar.activation(
                            outf[:], ps[:, 512:1024], Act.Identity, bias=bfb[:, ch : ch + 1]
                        )
                        nc.sync.dma_start(d_out[b, ch * 128 : ch * 128 + 128, tslf], outf[:])

                prev_ntp = None
                for tq in range(4):
                    tsl = slice(tq * 512, tq * 512 + 512)
                    ntp = mp.tile([128, 2, 512], bf16, tag="ntp", bufs=3, name=f"ntp{b}{tq}")
                    for h in range(H):
                        base = (h % 2) * 64
                        chq = h // 2
                        pso = pp.tile([128, 512], f32, tag="pso", bufs=2, name=f"pso{b}{tq}{h}")
                        for half in range(2):
                            pssc = pp.tile(
                                [128, 1024], f32, tag="sc", bufs=2, name=f"pssc{b}{tq}{h}{half}"
                            )
                            for j in range(2):
                                sb = half * 2 + j
                                kst = (
                                    krope[chq][base : base + 64, sb * 128 : sb * 128 + 128]
                                    .unsqueeze(1)
                                    .broadcast_to([64, 2, 128])
                                )
                                nc.tensor.matmul(
                                    pssc[:, j * 512 : j * 512 + 512],
                                    kst,
                                    qcs[chq][base : base + 64, :, tsl],
                                    start=True, stop=True, perf_mode=DR,
                                )
                            pr = mp.tile(
                                [128, 1024], bf16, tag="pr", bufs=6, name=f"pr{b}{tq}{h}{half}"
                            )
                            nc.scalar.activation(pr[:], pssc[:], Act.Exp, scale=0.125)
                            for j in range(2):
                                sc = half * 2 + j
                                nc.tensor.matmul(
                                    pso[:],
                                    vt[st][sc][:, h * 128 : h * 128 + 128],
                                    pr[:, j * 512 : j * 512 + 512],
                                    start=(sc == 0), stop=(sc == 3),
                                )
                        zr = mp.tile([64, 512], f32, tag="zr", bufs=4, name=f"zr{b}{tq}{h}")
                        nc.vector.reciprocal(zr[:], pso[64:128, :])
                        nc.vector.tensor_tensor(
                            ntp[base : base + 64, chq, :], pso[0:64, :], zr[:], Alu.mult
                        )
                        if h in (1, 3) and prev_ntp is not None:
                            emit_film(prev_ntp[0], prev_ntp[1], chs=(h // 2,))
                    prev_ntp = (ntp, tq)
                emit_film(prev_ntp[0], prev_ntp[1])

    nc.compile()
    return nc


def _host_prep(inputs):
    import ml_dtypes

    bf = ml_dtypes.bfloat16
    f8 = ml_dtypes.float8_e4m3

    wq, bq = inputs["wq"], inputs["bq"]
    wk, bk = inputs["wk"], inputs["bk"]
    wv, bv = inputs["wv"], inputs["bv"]
    wc, bc = inputs["w_cond"], inputs["b_cond"]
    wo = inputs["wo"]
    wf, bf_ = inputs["w_film"], inputs["b_film"]

    cosq, sinq = _rope_tables(TT)
    cosk, sink = _rope_tables(TS)
    wfo = (wf.astype(np.float64) @ wo.astype(np.float64)).astype(np.float32)
    b2 = (wf.astype(np.float64) @ inputs["bo"].astype(np.float64) + bf_).astype(np.float32)
    shared = {
        "cosq": cosq.astype(bf), "sinq": sinq.astype(bf),
        "cosk": cosk.astype(bf), "sink": sink.astype(bf),
        "wcT": _chunkT(wc, 4).astype(bf),
        "wqT": _chunkT(wq, 2).astype(f8),
        "wqrT": _chunkT(_rot_fold(wq), 2).astype(f8),
        "wkT": _chunkT(wk, 2).astype(bf),
        "wkrT": _chunkT(_rot_fold(wk), 2).astype(bf),
        "wvT": _chunkT(wv, 2).astype(bf),
        "bvT": np.ascontiguousarray(bv[None, :]).astype(bf),
        "wfoT": _chunkT(wfo, 2).astype(bf),
        "ident": np.eye(128, dtype=np.float32).astype(bf),
        "bcond": _colchunks(bc, 2),
        "bq": _colchunks(bq, 2),
        "bqr": _colchunks(_rot_fold(bq[:, None])[:, 0], 2),
        "bk": _colchunks(bk, 2),
        "bkr": _colchunks(_rot_fold(bk[:, None])[:, 0], 2),
        "bfg": _colchunks(b2[:HIDDEN], 2),
        "bfb": _colchunks(b2[HIDDEN:], 2),
    }
    return {k: np.ascontiguousarray(v) for k, v in shared.items()}


def kernel(**inputs):
    import ml_dtypes
    from concourse.bass_utils import run_bass_kernel_spmd

    bf = ml_dtypes.bfloat16
    f8 = ml_dtypes.float8_e4m3

    inputs = {k: np.asarray(v, dtype=np.float32) for k, v in inputs.items()}
    # masks are all-ones by problem spec (fill: ones); with ones masks the
    # reference's where()/final multiply are identities, so they are elided.

    if "nc" not in _CACHE:
        _CACHE["nc"] = _build_program()
    nc = _CACHE["nc"]

    shared = _host_prep(inputs)
    x = inputs["x"]
    cond = inputs["cond_latent"]
    in_maps = []
    for c in range(N_CORES):
        m = dict(shared)
        xs = x[c * BPC : (c + 1) * BPC]  # [BPC, 256, TT]
        # x8: [BPC, 128, 2, TT] fp8 planes (chunk kk on dim2)
        m["x8"] = np.ascontiguousarray(
            xs.reshape(BPC, 2, 128, TT).transpose(0, 2, 1, 3)
        ).astype(f8)
        m["xb"] = np.ascontiguousarray(
            xs.reshape(BPC, 2, 128, TT).transpose(0, 2, 1, 3)
        ).astype(bf)
        cs = cond[c * BPC : (c + 1) * BPC]
        m["cond"] = np.ascontiguousarray(
            cs.reshape(BPC, 4, 128, TS).transpose(0, 2, 1, 3)
        ).astype(bf)
        in_maps.append(m)

    res = run_bass_kernel_spmd(nc, in_maps, list(range(N_CORES)))
    out = np.concatenate([res.results[c]["out"] for c in range(N_CORES)], axis=0)
    return out.astype(np.float32)


# revision 11
# speedup vs baseline: 1.1106x; 1.0000x over previous
"""Trainium2 Bass kernel for nn_ConditioningEncoder (cross-attention conditioning
encoder: 1x1 convs + RoPE + 4-head cross-attention + output proj + FiLM).

Sharding: data-parallel over batch. B=16 across 8 cores -> 2 batch elements per
core. No collectives.

v3 design (vs the fp32r v2 baseline):
  - q/qr projections run as fp8e4 DoubleRow matmuls (two 128-row K-planes per
    instruction at 0.5 cycles/row); x is converted to fp8 on the host.
  - RoPE is folded into the scores matmul: qc = (q+bq)*cos and qs = (qr+bqr)*sin
    are stored as the two fp8 planes of a DoubleRow matmul whose stationary is
    the SAME k_rope block twice (stride-0 plane dim), so the PE computes
    k_rope^T(qc+qs) = scores with no rope-combine add and at 0.5 cycles/row.
  - exp() is one 4-bank-wide [128,2048] activation per (tq, head), writing bf16
    probabilities directly (no psum->fp32r rounding copies anywhere).
  - Each head's v^T stationary is [64 v-cols | 64 ones-cols], so the attention
    matmul (bf16, M=128) emits the attention output on psum rows 0:63 AND the
    softmax denominator Z broadcast across rows 64:127.  Normalize is then just
    reciprocal(psum->sbuf) + one tensor multiply (bf16 out).
  - FiLM: gamma/beta matmuls in bf16 share one 2-bank psum; tg=(gamma+bg)*x on
    DVE; tg is accumulated onto the beta psum with an identity-matmul; the
    final eviction is an ACT copy with the beta bias fused.
  - All conv biases are applied for free (STT scalar slots / ACT bias slots /
    one K=1 ones-row matmul for bv).  Masks are all-ones by problem spec, so
    the reference's where()/final multiply are identities and are elided.
  - DMA payloads are bf16/fp8 (host-converted); output returns fp32.
"""

import numpy as np

HIDDEN = 256
COND = 512
TT = 2048
TS = 512
H = 4
KC = 64
N_CORES = 8
B_FULL = 16
BPC = B_FULL // N_CORES  # batch elements per core

_CACHE = {}


def _rot_fold(w):
    """rotate_half as a signed row permutation applied to conv weight rows."""
    wr = np.empty_like(w)
    for h in range(H):
        b = KC * h
        wr[b : b + 32] = -w[b + 32 : b + 64]
        wr[b + 32 : b + 64] = w[b : b + 32]
    return wr


def _rope_tables(T):
    """Channel-major cos/sin tables [128, T]; rows repeat with period 64."""
    inv = 1.0 / (10000.0 ** (np.arange(0, KC, 2, dtype=np.float32) / KC))  # [32]
    t = np.arange(T, dtype=np.float32)
    f = t[None, :] * inv[:, None]  # [32, T]
    f64 = np.concatenate([f, f], 0)  # [64, T]
    f128 = np.concatenate([f64, f64], 0)  # [128, T]
    return np.cos(f128).astype(np.float32), np.sin(f128).astype(np.float32)


def _chunkT(w, n, p=128):
    """W [O, I] -> W.T chunked: [p, n, O] with [:, k, :] = W.T[p*k : p*(k+1), :]."""
    return np.ascontiguousarray(w.T.reshape(n, p, w.shape[0]).transpose(1, 0, 2))


def _colchunks(b, n, p=128):
    """bias [n*p] -> [p, n] with column m = chunk m."""
    return np.ascontiguousarray(b.reshape(n, p).T)


def _build_program():
    from concourse import bacc, mybir, tile

    dt = mybir.dt
    f32 = dt.float32
    bf16 = dt.bfloat16
    fp8 = dt.float8e4
    Alu = mybir.AluOpType
    Act = mybir.ActivationFunctionType
    DR = mybir.MatmulPerfMode.DoubleRow

    nc = bacc.Bacc(
        "TRN2",
        target_bir_lowering=False,
        debug=False,
        enable_asserts=False,
        num_devices=N_CORES,
    )

    d_x8 = nc.dram_tensor("x8", [BPC, 128, 2, TT], fp8, kind="ExternalInput")
    d_xb = nc.dram_tensor("xb", [BPC, 128, 2, TT], bf16, kind="ExternalInput")
    d_cond = nc.dram_tensor("cond", [BPC, 128, 4, TS], bf16, kind="ExternalInput")
    d_cosq = nc.dram_tensor("cosq", [128, TT], bf16, kind="ExternalInput")
    d_sinq = nc.dram_tensor("sinq", [128, TT], bf16, kind="ExternalInput")
    d_cosk = nc.dram_tensor("cosk", [128, TS], bf16, kind="ExternalInput")
    d_sink = nc.dram_tensor("sink", [128, TS], bf16, kind="ExternalInput")
    d_wcT = nc.dram_tensor("wcT", [128, 4, 256], bf16, kind="ExternalInput")
    d_wqT = nc.dram_tensor("wqT", [128, 2, 256], fp8, kind="ExternalInput")
    d_wqrT = nc.dram_tensor("wqrT", [128, 2, 256], fp8, kind="ExternalInput")
    d_wkT = nc.dram_tensor("wkT", [128, 2, 256], bf16, kind="ExternalInput")
    d_wkrT = nc.dram_tensor("wkrT", [128, 2, 256], bf16, kind="ExternalInput")
    d_wvT = nc.dram_tensor("wvT", [128, 2, 256], bf16, kind="ExternalInput")
    d_bvT = nc.dram_tensor("bvT", [1, 256], bf16, kind="ExternalInput")
    d_wfoT = nc.dram_tensor("wfoT", [128, 2, 512], bf16, kind="ExternalInput")
    d_ident = nc.dram_tensor("ident", [128, 128], bf16, kind="ExternalInput")
    # biases as [128, n] column chunks (fp32: STT scalar / ACT bias operands)
    d_bcond = nc.dram_tensor("bcond", [128, 2], f32, kind="ExternalInput")
    d_bq = nc.dram_tensor("bq", [128, 2], f32, kind="ExternalInput")
    d_bqr = nc.dram_tensor("bqr", [128, 2], f32, kind="ExternalInput")
    d_bk = nc.dram_tensor("bk", [128, 2], f32, kind="ExternalInput")
    d_bkr = nc.dram_tensor("bkr", [128, 2], f32, kind="ExternalInput")
    d_bfg = nc.dram_tensor("bfg", [128, 2], f32, kind="ExternalInput")
    d_bfb = nc.dram_tensor("bfb", [128, 2], f32, kind="ExternalInput")
    d_out = nc.dram_tensor("out", [BPC, HIDDEN, TT], f32, kind="ExternalOutput")

    with tile.TileContext(nc) as tc:
        with (
            tc.tile_pool(name="wp", bufs=1) as wp,
            tc.tile_pool(name="mp", bufs=2) as mp,
            tc.tile_pool(name="pp", bufs=1, space="PSUM") as pp,
        ):
            # ---- persistent tables / weights ----
            cosq = wp.tile([128, TT], bf16)
            sinq = wp.tile([128, TT], bf16)
            cosk = wp.tile([128, TS], bf16)
            sink = wp.tile([128, TS], bf16)
            wcT = wp.tile([128, 4, 256], bf16)
            wqT = wp.tile([128, 2, 256], fp8)
            wqrT = wp.tile([128, 2, 256], fp8)
            wkT = wp.tile([128, 2, 256], bf16)
            wkrT = wp.tile([128, 2, 256], bf16)
            wvT = wp.tile([128, 2, 256], bf16)
            bvT = wp.tile([1, 256], bf16)
            wfoT = wp.tile([128, 2, 512], bf16)
            ident = wp.tile([128, 128], bf16)
            bcond = wp.tile([128, 2], f32)
            bq = wp.tile([128, 2], f32)
            bqr = wp.tile([128, 2], f32)
            bk = wp.tile([128, 2], f32)
            bkr = wp.tile([128, 2], f32)
            bfg = wp.tile([128, 2], f32)
            bfb = wp.tile([128, 2], f32)
            for t, d in [
                (cosq, d_cosq), (sinq, d_sinq), (cosk, d_cosk), (sink, d_sink),
                (wcT, d_wcT), (wqT, d_wqT), (wqrT, d_wqrT), (wkT, d_wkT),
                (wkrT, d_wkrT), (wvT, d_wvT), (bvT, d_bvT), (wfoT, d_wfoT),
                (ident, d_ident), (bcond, d_bcond), (bq, d_bq), (bqr, d_bqr),
                (bk, d_bk), (bkr, d_bkr), (bfg, d_bfg), (bfb, d_bfb),
            ]:
                nc.sync.dma_start(t[:], d[:])
            ones1 = wp.tile([1, 128], bf16)
            ones1f = wp.tile([1, 128], f32)
            nc.vector.memset(ones1f[:], 1.0)
            nc.vector.tensor_copy(ones1[:], ones1f[:])
            # persistent per-head [64 v | 64 ones] stationaries; ones prefilled
            onesw = wp.tile([128, 256], f32)
            nc.vector.memset(onesw[:], 1.0)
            vt = [[wp.tile([128, 512], bf16, name=f"vt{_s}{_c}") for _c in range(4)] for _s in range(2)]
            for st in range(2):
                for sc in range(4):
                    nc.vector.tensor_copy(
                        vt[st][sc][:].rearrange("p (h c) -> p h c", h=4, c=128)[:, :, 64:128],
                        onesw[:].rearrange("p (h c) -> p h c", h=4, c=64),
                    )

            for b in range(BPC):
                st = b % 2
                # ---- loads ----
                x8 = mp.tile([128, 2, TT], fp8, tag="x8", bufs=2, name=f"x8_{b}")
                nc.sync.dma_start(x8[:], d_x8[b])
                xb = mp.tile([128, 2, TT], bf16, tag="xb", bufs=2, name=f"xb_{b}")
                nc.sync.dma_start(xb[:], d_xb[b])
                cb = mp.tile([128, 4, TS], bf16, tag="cond", bufs=2, name=f"cond_{b}")
                nc.sync.dma_start(cb[:], d_cond[b])

                # ---- c = w_cond @ cond + b_cond  (bf16, evict on ACT w/ bias) ----
                c_sb = mp.tile([128, 2, TS], bf16, tag="c", bufs=3, name=f"c_{b}")
                for m in range(2):
                    ps = pp.tile([128, 1024], f32, tag="gen", bufs=1, name=f"psc{b}{m}")
                    for kk in range(4):
                        nc.tensor.matmul(
                            ps[:, 0:512],
                            wcT[:, kk, m * 128 : m * 128 + 128],
                            cb[:, kk, :],
                            start=(kk == 0),
                            stop=(kk == 3),
                        )
                    nc.scalar.activation(
                        c_sb[:, m, :], ps[:, 0:512], Act.Identity, bias=bcond[:, m : m + 1]
                    )

                # ---- k/kr + rope -> krope fp8 (STT on DVE, add on Pool) ----
                krope = []
                for m in range(2):
                    ps = pp.tile([128, 1024], f32, tag="gen", bufs=1, name=f"psk{b}{m}")
                    for kk in range(2):
                        nc.tensor.matmul(
                            ps[:, 0:512], wkT[:, kk, m * 128 : m * 128 + 128],
                            c_sb[:, kk, :], start=(kk == 0), stop=(kk == 1),
                        )
                    for kk in range(2):
                        nc.tensor.matmul(
                            ps[:, 512:1024], wkrT[:, kk, m * 128 : m * 128 + 128],
                            c_sb[:, kk, :], start=(kk == 0), stop=(kk == 1),
                        )
                    kc = mp.tile([128, TS], bf16, tag="kc", bufs=3, name=f"kc{b}{m}")
                    ks = mp.tile([128, TS], bf16, tag="ks", bufs=3, name=f"ks{b}{m}")
                    nc.vector.scalar_tensor_tensor(
                        kc[:], ps[:, 0:512], bk[:, m : m + 1], cosk[:],
                        op0=Alu.add, op1=Alu.mult,
                    )
                    nc.vector.scalar_tensor_tensor(
                        ks[:], ps[:, 512:1024], bkr[:, m : m + 1], sink[:],
                        op0=Alu.add, op1=Alu.mult,
                    )
                    kr = mp.tile([128, TS], fp8, tag="krope", bufs=4, name=f"krope{b}{m}")
                    nc.gpsimd.tensor_tensor(kr[:], kc[:], ks[:], Alu.add)
                    krope.append(kr)

                # ---- v^T (+bias via ones-row matmul), evict strided into vt ----
                for sc in range(4):
                    ps = pp.tile([128, 1024], f32, tag="gen", bufs=1, name=f"psv{b}{sc}")
                    po = ps[:, 0:256]
                    for kk in range(2):
                        nc.tensor.matmul(
                            po, c_sb[:, kk, sc * 128 : sc * 128 + 128],
                            wvT[:, kk, :], start=(kk == 0), stop=False,
                        )
                    nc.tensor.matmul(po, ones1[0:1, :], bvT[0:1, :], start=False, stop=True)
                    nc.scalar.activation(
                        vt[st][sc][:].rearrange("p (h c) -> p h c", h=4, c=128)[:, :, 0:64],
                        po.rearrange("p (h c) -> p h c", h=4, c=64),
                        Act.Copy,
                    )

                # ---- q/qr + rope -> qcs fp8 planes (DR matmuls; wide STTs) ----
                qcs = []
                for m in range(2):
                    qt = mp.tile([128, 2, TT], fp8, tag="qcs", bufs=4, name=f"qcs{b}{m}")
                    qcs.append(qt)
                for m in range(2):
                    for nb4 in range(4):
                        nb = slice(nb4 * 512, nb4 * 512 + 512)
                        ps = pp.tile([128, 1024], f32, tag="sc", bufs=2, name=f"psq{b}{m}{nb4}")
                        nc.tensor.matmul(
                            ps[:, 0:512], wqT[:, :, m * 128 : m * 128 + 128],
                            x8[:, :, nb], start=True, stop=True, perf_mode=DR,
                        )
                        nc.tensor.matmul(
                            ps[:, 512:1024], wqrT[:, :, m * 128 : m * 128 + 128],
                            x8[:, :, nb], start=True, stop=True, perf_mode=DR,
                        )
                        nc.vector.scalar_tensor_tensor(
                            qcs[m][:, 0, nb], ps[:, 0:512], bq[:, m : m + 1],
                            cosq[:, nb], op0=Alu.add, op1=Alu.mult,
                        )
                        nc.vector.scalar_tensor_tensor(
                            qcs[m][:, 1, nb], ps[:, 512:1024], bqr[:, m : m + 1],
                            sinq[:, nb], op0=Alu.add, op1=Alu.mult,
                        )

                # ---- attention + film per t-quarter (film delayed one tq and
                # interleaved between heads so the PE never waits on the film
                # eviction chain) ----
                def emit_film(ntp_t, tq_f, chs=(0, 1)):
                    tslf = slice(tq_f * 512, tq_f * 512 + 512)
                    for ch in chs:
                        ps = pp.tile([128, 1024], f32, tag="gen", bufs=1, name=f"psf{b}{tq_f}{ch}")
                        for kk in range(2):
                            nc.tensor.matmul(
                                ps[:, 0:512], wfoT[:, kk, ch * 128 : ch * 128 + 128],
                                ntp_t[:, kk, :], start=(kk == 0), stop=(kk == 1),
                            )
                        for kk in range(2):
                            nc.tensor.matmul(
                                ps[:, 512:1024],
                                wfoT[:, kk, (ch + 2) * 128 : (ch + 2) * 128 + 128],
                                ntp_t[:, kk, :], start=(kk == 0), stop=False,
                            )
                        tg = mp.tile([128, 512], bf16, tag="tg", bufs=4, name=f"tg{b}{tq_f}{ch}")
                        nc.vector.scalar_tensor_tensor(
                            tg[:], ps[:, 0:512], bfg[:, ch : ch + 1],
                            xb[:, ch, tslf], op0=Alu.add, op1=Alu.mult,
                        )
                        nc.tensor.matmul(ps[:, 512:1024], ident[:], tg[:], start=False, stop=True)
                        outf = mp.tile([128, 512], f32, tag="outf", bufs=4, name=f"o{b}{tq_f}{ch}")
                        nc.scalar.activation(
                            outf[:], ps[:, 512:1024], Act.Identity, bias=bfb[:, ch : ch + 1]
                        )
                        nc.sync.dma_start(d_out[b, ch * 128 : ch * 128 + 128, tslf], outf[:])

                prev_ntp = None
                for tq in range(4):
                    tsl = slice(tq * 512, tq * 512 + 512)
                    ntp = mp.tile([128, 2, 512], bf16, tag="ntp", bufs=3, name=f"ntp{b}{tq}")
                    for h in range(H):
                        base = (h % 2) * 64
                        chq = h // 2
                        pso = pp.tile([128, 512], f32, tag="pso", bufs=2, name=f"pso{b}{tq}{h}")
                        for half in range(2):
                            pssc = pp.tile(
                                [128, 1024], f32, tag="sc", bufs=2, name=f"pssc{b}{tq}{h}{half}"
                            )
                            for j in range(2):
                                sb = half * 2 + j
                                kst = (
                                    krope[chq][base : base + 64, sb * 128 : sb * 128 + 128]
                                    .unsqueeze(1)
                                    .broadcast_to([64, 2, 128])
                                )
                                nc.tensor.matmul(
                                    pssc[:, j * 512 : j * 512 + 512],
                                    kst,
                                    qcs[chq][base : base + 64, :, tsl],
                                    start=True, stop=True, perf_mode=DR,
                                )
                            pr = mp.tile(
                                [128, 1024], bf16, tag="pr", bufs=8, name=f"pr{b}{tq}{h}{half}"
                            )
                            nc.scalar.activation(pr[:], pssc[:], Act.Exp, scale=0.125)
                            for j in range(2):
                                sc = half * 2 + j
                                nc.tensor.matmul(
                                    pso[:],
                                    vt[st][sc][:, h * 128 : h * 128 + 128],
                                    pr[:, j * 512 : j * 512 + 512],
                                    start=(sc == 0), stop=(sc == 3),
                                )
                        zr = mp.tile([64, 512], f32, tag="zr", bufs=4, name=f"zr{b}{tq}{h}")
                        nc.vector.reciprocal(zr[:], pso[64:128, :])
                        nc.vector.tensor_tensor(
                            ntp[base : base + 64, chq, :], pso[0:64, :], zr[:], Alu.mult
                        )
                        if h in (1, 3) and prev_ntp is not None:
                            emit_film(prev_ntp[0], prev_ntp[1], chs=(h // 2,))
                    prev_ntp = (ntp, tq)
                emit_film(prev_ntp[0], prev_ntp[1])

    nc.compile()
    return nc


def _host_prep(inputs):
    import ml_dtypes

    bf = ml_dtypes.bfloat16
    f8 = ml_dtypes.float8_e4m3

    wq, bq = inputs["wq"], inputs["bq"]
    wk, bk = inputs["wk"], inputs["bk"]
    wv, bv = inputs["wv"], inputs["bv"]
    wc, bc = inputs["w_cond"], inputs["b_cond"]
    wo = inputs["wo"]
    wf, bf_ = inputs["w_film"], inputs["b_film"]

    cosq, sinq = _rope_tables(TT)
    cosk, sink = _rope_tables(TS)
    wfo = (wf.astype(np.float64) @ wo.astype(np.float64)).astype(np.float32)
    b2 = (wf.astype(np.float64) @ inputs["bo"].astype(np.float64) + bf_).astype(np.float32)
    shared = {
        "cosq": cosq.astype(bf), "sinq": sinq.astype(bf),
        "cosk": cosk.astype(bf), "sink": sink.astype(bf),
        "wcT": _chunkT(wc, 4).astype(bf),
        "wqT": _chunkT(wq, 2).astype(f8),
        "wqrT": _chunkT(_rot_fold(wq), 2).astype(f8),
        "wkT": _chunkT(wk, 2).astype(bf),
        "wkrT": _chunkT(_rot_fold(wk), 2).astype(bf),
        "wvT": _chunkT(wv, 2).astype(bf),
        "bvT": np.ascontiguousarray(bv[None, :]).astype(bf),
        "wfoT": _chunkT(wfo, 2).astype(bf),
        "ident": np.eye(128, dtype=np.float32).astype(bf),
        "bcond": _colchunks(bc, 2),
        "bq": _colchunks(bq, 2),
        "bqr": _colchunks(_rot_fold(bq[:, None])[:, 0], 2),
        "bk": _colchunks(bk, 2),
        "bkr": _colchunks(_rot_fold(bk[:, None])[:, 0], 2),
        "bfg": _colchunks(b2[:HIDDEN], 2),
        "bfb": _colchunks(b2[HIDDEN:], 2),
    }
    return {k: np.ascontiguousarray(v) for k, v in shared.items()}


def kernel(**inputs):
    import ml_dtypes
    from concourse.bass_utils import run_bass_kernel_spmd

    bf = ml_dtypes.bfloat16
    f8 = ml_dtypes.float8_e4m3

    inputs = {k: np.asarray(v, dtype=np.float32) for k, v in inputs.items()}
    # masks are all-ones by problem spec (fill: ones); with ones masks the
    # reference's where()/final multiply are identities, so they are elided.

    if "nc" not in _CACHE:
        _CACHE["nc"] = _build_program()
    nc = _CACHE["nc"]

    shared = _host_prep(inputs)
    x = inputs["x"]
    cond = inputs["cond_latent"]
    in_maps = []
    for c in range(N_CORES):
        m = dict(shared)
        xs = x[c * BPC : (c + 1) * BPC]  # [BPC, 256, TT]
        # x8: [BPC, 128, 2, TT] fp8 planes (chunk kk on dim2)
        m["x8"] = np.ascontiguousarray(
            xs.reshape(BPC, 2, 128, TT).transpose(0, 2, 1, 3)
        ).astype(f8)
        m["xb"] = np.ascontiguousarray(
            xs.reshape(BPC, 2, 128, TT).transpose(0, 2, 1, 3)
        ).astype(bf)
        cs = cond[c * BPC : (c + 1) * BPC]
        m["cond"] = np.ascontiguousarray(
            cs.reshape(BPC, 4, 128, TS).transpose(0, 2, 1, 3)
        ).astype(bf)
        in_maps.append(m)

    res = run_bass_kernel_spmd(nc, in_maps, list(range(N_CORES)))
    out = np.concatenate([res.results[c]["out"] for c in range(N_CORES)], axis=0)
    return out.astype(np.float32)
